# revision 25
# baseline (speedup 1.0000x reference)
"""Multi-head causal attention (B=2,S=2048,D=1024,H=16,DH=64) on 8 TRN2 cores.

Sharding: 2 heads per core (tensor parallel). Each core computes QKV for its
2 heads from the full x, causal attention, and its partial of the output
projection [B,S,D]. The host sums the 8 partials (the W_O head-sum).

On-device layouts (matmul contracts over the partition dim):
  QT/KT  [2*DH=128 part, S]   (heads stacked on partitions; 1/sqrt(DH) folded into W_Q)
  V      [S part (128-blocks), heads, DH+1]  (ones column -> softmax row-sums for free)
  S^T    [k 128 part, q 512]  per (k-block, q-tile); above-diagonal blocks skipped
  Z'^T   [DH+1 part, q 512]   accumulated over k-blocks; row DH = exp row-sum
  out    partial [B,S,D] bf16, summed across cores on host
"""

import os
import sys

import numpy as np

if "/opt/trn_rl_repo" not in sys.path:
    sys.path.insert(0, "/opt/trn_rl_repo")

import ml_dtypes

B, S, D, H, DH = 2, 2048, 1024, 16, 64
NCORES = 8
HPC = H // NCORES          # heads per core
P = 128
QT_W = 512                 # q-tile width
NQT = S // QT_W            # 4 q-tiles
NKB = S // P               # 16 k-blocks
NDC = D // P               # 8 contraction chunks for projections
NEG = -1.0e5

BF16 = ml_dtypes.bfloat16

_CACHE = {}


def _build_nc(B=B, S=S, D=D, HPC=HPC, DH=DH):
    import concourse.tile as tile
    import concourse.mybir as mybir
    from concourse import bacc
    from concourse import masks
    from contextlib import ExitStack

    QT_W = 512
    NQT = S // QT_W
    NKB = S // P
    NDC = D // P

    f32 = mybir.dt.float32
    bf16 = mybir.dt.bfloat16
    AF = mybir.ActivationFunctionType
    ALU = mybir.AluOpType

    nc = bacc.Bacc("TRN2", target_bir_lowering=False, debug=False,
                   num_devices=NCORES)

    xT = nc.dram_tensor("xT", [B, D, S], bf16, kind="ExternalInput").ap()
    wq_d = nc.dram_tensor("wq", [D, HPC * DH], bf16, kind="ExternalInput").ap()
    wk_d = nc.dram_tensor("wk", [D, HPC * DH], bf16, kind="ExternalInput").ap()
    wv_d = nc.dram_tensor("wv", [D, HPC * DH], bf16, kind="ExternalInput").ap()
    wo_d = nc.dram_tensor("wo", [HPC * DH, D], bf16, kind="ExternalInput").ap()
    bq_d = nc.dram_tensor("bq", [HPC * DH, 1], f32, kind="ExternalInput").ap()
    bk_d = nc.dram_tensor("bk", [HPC * DH, 1], f32, kind="ExternalInput").ap()
    msk_d = nc.dram_tensor("msk", [P, P], f32, kind="ExternalInput").ap()
    out_d = nc.dram_tensor("out", [B, S, D], bf16, kind="ExternalOutput").ap()

    with tile.TileContext(nc) as tc, ExitStack() as ctx:
        const = ctx.enter_context(tc.tile_pool(name="const", bufs=1))
        qk_pool = ctx.enter_context(tc.tile_pool(name="qk", bufs=4))
        v_pool = ctx.enter_context(tc.tile_pool(name="v", bufs=2))
        pt_pool = ctx.enter_context(tc.tile_pool(name="pt", bufs=8))
        sm_pool = ctx.enter_context(tc.tile_pool(name="sm", bufs=6))
        zt_pool = ctx.enter_context(tc.tile_pool(name="zt", bufs=4))
        o_pool = ctx.enter_context(tc.tile_pool(name="o", bufs=3))
        st_ps = ctx.enter_context(tc.tile_pool(name="stps", bufs=3, space="PSUM"))
        z_ps = ctx.enter_context(tc.tile_pool(name="zps", bufs=2, space="PSUM"))
        rb_ps = ctx.enter_context(tc.tile_pool(name="rbps", bufs=1, space="PSUM"))
        mm_ps = ctx.enter_context(tc.tile_pool(name="mmps", bufs=2, space="PSUM"))

        # ---- resident constants ----
        # weights first (small, unblock the first projections), then x^T for
        # batch 0 split across two DMA queues, then batch 1.
        wq_sb = const.tile([P, NDC, HPC * DH], bf16)
        nc.sync.dma_start(wq_sb[:], wq_d.rearrange("(dc p) m -> p dc m", p=P))
        wk_sb = const.tile([P, NDC, HPC * DH], bf16)
        nc.gpsimd.dma_start(wk_sb[:], wk_d.rearrange("(dc p) m -> p dc m", p=P))
        wv_sb = const.tile([P, NDC, HPC * DH], bf16)
        nc.scalar.dma_start(wv_sb[:], wv_d.rearrange("(dc p) m -> p dc m", p=P))
        wo_sb = const.tile([HPC * DH, D], bf16)
        nc.scalar.dma_start(wo_sb[:], wo_d[:])
        bq_sb = const.tile([HPC * DH, 1], f32)
        nc.scalar.dma_start(bq_sb[:], bq_d[:])
        bk_sb = const.tile([HPC * DH, 1], f32)
        nc.scalar.dma_start(bk_sb[:], bk_d[:])
        msk_sb = const.tile([P, P], f32)
        nc.scalar.dma_start(msk_sb[:], msk_d[:])
        ones_sb = const.tile([1, DH], bf16)
        nc.vector.memset(ones_sb[:], 1.0)
        xt_sb = const.tile([P, B, NDC, S], bf16)
        _qs = (nc.sync, nc.gpsimd, nc.scalar)
        for b in range(B):
            for dc in range(NDC):
                _qs[dc % 3].dma_start(
                    xt_sb[:, b, dc, :],
                    xT[b, dc * P:(dc + 1) * P, :])

        qt = {}
        kt = {}
        vv = {}

        def qkv_chunk(b, t):
            """Q and K projection for q-tile t of batch b (PE-dense filler)."""
            for w_sb, dst, bias in ((wq_sb, qt[b], bq_sb),
                                    (wk_sb, kt[b], bk_sb)):
                ps = mm_ps.tile([P, QT_W], f32, tag="mm")
                for dc in range(NDC):
                    nc.tensor.matmul(
                        ps[:], w_sb[:, dc, :],
                        xt_sb[:, b, dc, t * QT_W:(t + 1) * QT_W],
                        start=(dc == 0), stop=(dc == NDC - 1))
                nc.vector.tensor_tensor(
                    dst[:, t * QT_W:(t + 1) * QT_W], ps[:],
                    bias[:].to_broadcast([P, QT_W]), ALU.add)

        def v_chunk(b, g):
            """V projection for s-blocks 2g..2g+1 of batch b (one copy)."""
            ps = mm_ps.tile([P, QT_W], f32, tag="mm")
            for i in range(2):
                sb = 2 * g + i
                for dc in range(NDC):
                    nc.tensor.matmul(
                        ps[:, i * P:i * P + HPC * DH],
                        xt_sb[:, b, dc, sb * P:(sb + 1) * P],
                        wv_sb[:, dc, :],
                        start=(dc == 0), stop=(dc == NDC - 1),
                        skip_group_check=True)
            nc.vector.tensor_copy(
                out=vv[b][:, 2 * g:2 * g + 2, :, 0:DH],
                in_=ps[:, 0:2 * P].rearrange("p (s h e) -> p s h e", h=HPC, e=DH))

        def attn_unit(b, h, t, zt_sb):
            """Scores + softmax + AV for one (batch, head, q-tile).

            Software-pipelined by 2: the PE queue sees S(kb+1), S(kb+2)
            ahead of AV(kb), so the exp wait never blocks score matmuls."""
            qt_sb, kt_sb, v_sb = qt[b], kt[b], vv[b]
            nkb = 4 * t + 4
            DEPTH = 2
            zps = z_ps.tile([P, QT_W], f32, tag="z")
            pending = []

            def emit_scores(kb):
                j = kb - 4 * t  # >=0 -> diagonal-region block
                width = QT_W - P * j if j >= 0 else QT_W
                qoff = P * j if j >= 0 else 0
                sps = st_ps.tile([P, QT_W], f32, tag="st")
                nc.tensor.matmul(
                    sps[:, 0:width],
                    kt_sb[h * DH:(h + 1) * DH, kb * P:(kb + 1) * P],
                    qt_sb[h * DH:(h + 1) * DH,
                          t * QT_W + qoff:(t + 1) * QT_W],
                    start=True, stop=True)
                if j >= 0:
                    nc.vector.tensor_tensor(
                        sps[:, 0:P], sps[:, 0:P], msk_sb[:], ALU.add)
                pt = pt_pool.tile([P, QT_W], bf16, tag="pt")
                nc.scalar.activation(pt[:, 0:width], sps[:, 0:width], AF.Exp)
                return (kb, pt, width, qoff)

            def emit_av(kb, pt, width, qoff):
                nc.tensor.matmul(
                    zps[0:DH + 1, qoff:QT_W],
                    v_sb[:, kb, h, :],
                    pt[:, 0:width],
                    start=(kb == 0), stop=(kb == nkb - 1),
                    skip_group_check=True)

            for kb in range(nkb):
                pending.append(emit_scores(kb))
                if len(pending) > DEPTH:
                    emit_av(*pending.pop(0))
            for item in pending:
                emit_av(*item)
            # normalize: Z = Z' * (1/rowsum); rowsum lives in zps row DH.
            # Broadcast rowsum over rows DH..127 of the same PSUM tile via a
            # K=1 matmul, then fast-reciprocal and multiply.
            rs_sb = sm_pool.tile([1, QT_W], bf16, tag="rs")
            nc.vector.tensor_copy(out=rs_sb[:], in_=zps[DH:DH + 1, :])
            rbps = rb_ps.tile([DH, QT_W], f32, tag="rb")
            nc.tensor.matmul(rbps[:], ones_sb[:], rs_sb[:],
                             start=True, stop=True)
            rc_sb = sm_pool.tile([DH, QT_W], f32, tag="rc")
            nc.vector.reciprocal_approx_fast(out=rc_sb[:], in_=rbps[:])
            nc.vector.tensor_tensor(
                zt_sb[h * DH:(h + 1) * DH, :], zps[0:DH, :], rc_sb[:],
                ALU.mult)

        def oproj(b, t, zt_sb):
            for c in range(QT_W // P):
                o_sb = o_pool.tile([P, D], bf16, tag="o")
                for half in range(2):
                    ops = mm_ps.tile([P, QT_W], f32, tag="mm")
                    nc.tensor.matmul(
                        ops[:], zt_sb[:, c * P:(c + 1) * P],
                        wo_sb[:, half * 512:(half + 1) * 512],
                        start=True, stop=True)
                    nc.any.tensor_copy(
                        out=o_sb[:, half * 512:(half + 1) * 512],
                        in_=ops[:])
                row0 = t * QT_W + c * P
                nc.sync.dma_start(out_d[b, row0:row0 + P, :], o_sb[:])

        for b in range(B):
            qt[b] = qk_pool.tile([P, S], bf16, tag="qt", name=f"qt{b}")
            kt[b] = qk_pool.tile([P, S], bf16, tag="qt", name=f"kt{b}")
            vv[b] = v_pool.tile([P, NKB, HPC, DH + 1], bf16, tag="v", name=f"v{b}")
            nc.vector.memset(vv[b][:, :, :, DH:DH + 1], 1.0)

        # batch 0 projections up front (dense PE work, warms HAM)
        for t in range(NQT):
            qkv_chunk(0, t)
        for g in range(NKB // 2):
            v_chunk(0, g)

        # batch-0 attention interleaved with batch-1 Q/K projections so the
        # PE has dense filler while ScalarE runs the exps
        for t in range(NQT):
            zt_sb = zt_pool.tile([P, QT_W], bf16, tag="zt")
            for h in range(HPC):
                u = t * HPC + h
                attn_unit(0, h, t, zt_sb)
                if u < NQT:
                    qkv_chunk(1, u)
                elif u == NQT:
                    v_chunk(1, 0)
                    v_chunk(1, 1)
            oproj(0, t, zt_sb)

        # batch-1 attention; remaining V blocks emitted just-in-time per
        # q-tile so they act as PE filler between the exp-bound units
        for t in range(NQT):
            if t > 0:
                v_chunk(1, 2 * t)
                v_chunk(1, 2 * t + 1)
            zt_sb = zt_pool.tile([P, QT_W], bf16, tag="zt")
            for h in range(HPC):
                attn_unit(1, h, t, zt_sb)
            oproj(1, t, zt_sb)

    nc.compile()
    return nc


def _prep_in_maps(inputs):
    x = np.asarray(inputs["x"], dtype=np.float32)
    xT = np.ascontiguousarray(x.transpose(0, 2, 1)).astype(BF16)  # [B, D, S]
    W_Q = np.asarray(inputs["W_Q"], dtype=np.float32)
    W_K = np.asarray(inputs["W_K"], dtype=np.float32)
    W_V = np.asarray(inputs["W_V"], dtype=np.float32)
    W_O = np.asarray(inputs["W_O"], dtype=np.float32)
    b_Q = np.asarray(inputs["b_Q"], dtype=np.float32)
    b_K = np.asarray(inputs["b_K"], dtype=np.float32)
    scale = 1.0 / np.sqrt(DH)
    msk = np.where(np.arange(P)[:, None] <= np.arange(P)[None, :],
                   np.float32(0.0), np.float32(NEG)).astype(np.float32)
    in_maps = []
    for c in range(NCORES):
        hs = [HPC * c + i for i in range(HPC)]
        wq = np.concatenate([W_Q[h] for h in hs], axis=1) * scale
        wk = np.concatenate([W_K[h] for h in hs], axis=1)
        wv = np.concatenate([W_V[h] for h in hs], axis=1)
        wo = np.concatenate([W_O[h] for h in hs], axis=0)
        bq = np.concatenate([b_Q[h] for h in hs])[:, None] * scale
        bk = np.concatenate([b_K[h] for h in hs])[:, None]
        in_maps.append({
            "xT": xT,
            "wq": np.ascontiguousarray(wq).astype(BF16),
            "wk": np.ascontiguousarray(wk).astype(BF16),
            "wv": np.ascontiguousarray(wv).astype(BF16),
            "wo": np.ascontiguousarray(wo).astype(BF16),
            "bq": bq.astype(np.float32),
            "bk": bk.astype(np.float32),
            "msk": msk,
        })
    return in_maps


def _run(inputs, trace=False, trace_cores=None):
    from concourse.bass_utils import run_bass_kernel_spmd

    if "nc" not in _CACHE:
        _CACHE["nc"] = _build_nc()
    nc = _CACHE["nc"]
    in_maps = _prep_in_maps(inputs)
    res = run_bass_kernel_spmd(
        nc, in_maps, core_ids=list(range(NCORES)),
        trace=trace, trace_cores=trace_cores)

    out = np.zeros((B, S, D), dtype=np.float32)
    for c in range(NCORES):
        out += res.results[c]["out"].astype(np.float32)
    # exact host fold of the zero-pattern-sum bias terms:
    # z includes +b_V per head -> out += sum_h b_V[h] @ W_O[h]; plus b_O.
    b_V = np.asarray(inputs["b_V"], dtype=np.float32)
    W_O = np.asarray(inputs["W_O"], dtype=np.float32)
    b_O = np.asarray(inputs["b_O"], dtype=np.float32)
    out += np.einsum("he,hed->d", b_V, W_O) + b_O

    residual = np.asarray(inputs["residual"], dtype=np.float32)
    return (residual, out), res


def kernel(**inputs):
    (residual, out), _ = _run(inputs, trace=False)
    return residual, out


# revision 26
# speedup vs baseline: 1.0384x; 1.0384x over previous
"""Multi-head causal attention (B=2,S=2048,D=1024,H=16,DH=64) on 8 TRN2 cores.

Sharding: 2 heads per core (tensor parallel). Each core computes QKV for its
2 heads from the full x, causal attention, and its partial of the output
projection [B,S,D]. The host sums the 8 partials (the W_O head-sum).

On-device layouts (matmul contracts over the partition dim):
  QT/KT  [2*DH=128 part, S]   (heads stacked on partitions; 1/sqrt(DH) folded into W_Q)
  V      [S part (128-blocks), heads, DH+1]  (ones column -> softmax row-sums for free)
  S^T    [k 128 part, q 512]  per (k-block, q-tile); above-diagonal blocks skipped
  Z'^T   [DH+1 part, q 512]   accumulated over k-blocks; row DH = exp row-sum
  out    partial [B,S,D] bf16, summed across cores on host
"""

import os
import sys

import numpy as np

if "/opt/trn_rl_repo" not in sys.path:
    sys.path.insert(0, "/opt/trn_rl_repo")

import ml_dtypes

B, S, D, H, DH = 2, 2048, 1024, 16, 64
NCORES = 8
HPC = H // NCORES          # heads per core
P = 128
QT_W = 512                 # q-tile width
NQT = S // QT_W            # 4 q-tiles
NKB = S // P               # 16 k-blocks
NDC = D // P               # 8 contraction chunks for projections
NEG = -1.0e5

BF16 = ml_dtypes.bfloat16

_CACHE = {}


def _build_nc(B=B, S=S, D=D, HPC=HPC, DH=DH):
    import concourse.tile as tile
    import concourse.mybir as mybir
    from concourse import bacc
    from concourse import masks
    from contextlib import ExitStack

    QT_W = 512
    NQT = S // QT_W
    NKB = S // P
    NDC = D // P

    f32 = mybir.dt.float32
    bf16 = mybir.dt.bfloat16
    AF = mybir.ActivationFunctionType
    ALU = mybir.AluOpType

    nc = bacc.Bacc("TRN2", target_bir_lowering=False, debug=False,
                   num_devices=NCORES)

    xT = nc.dram_tensor("xT", [B, D, S], bf16, kind="ExternalInput").ap()
    wq_d = nc.dram_tensor("wq", [D, HPC * DH], bf16, kind="ExternalInput").ap()
    wk_d = nc.dram_tensor("wk", [D, HPC * DH], bf16, kind="ExternalInput").ap()
    wv_d = nc.dram_tensor("wv", [D, HPC * DH], bf16, kind="ExternalInput").ap()
    wo_d = nc.dram_tensor("wo", [HPC * DH, D], bf16, kind="ExternalInput").ap()
    bq_d = nc.dram_tensor("bq", [HPC * DH, 1], f32, kind="ExternalInput").ap()
    bk_d = nc.dram_tensor("bk", [HPC * DH, 1], f32, kind="ExternalInput").ap()
    msk_d = nc.dram_tensor("msk", [P, P], f32, kind="ExternalInput").ap()
    out_d = nc.dram_tensor("out", [B, S, D], bf16, kind="ExternalOutput").ap()

    with tile.TileContext(nc) as tc, ExitStack() as ctx:
        const = ctx.enter_context(tc.tile_pool(name="const", bufs=1))
        qk_pool = ctx.enter_context(tc.tile_pool(name="qk", bufs=4))
        v_pool = ctx.enter_context(tc.tile_pool(name="v", bufs=2))
        pt_pool = ctx.enter_context(tc.tile_pool(name="pt", bufs=8))
        sm_pool = ctx.enter_context(tc.tile_pool(name="sm", bufs=6))
        zt_pool = ctx.enter_context(tc.tile_pool(name="zt", bufs=4))
        o_pool = ctx.enter_context(tc.tile_pool(name="o", bufs=3))
        st_ps = ctx.enter_context(tc.tile_pool(name="stps", bufs=3, space="PSUM"))
        z_ps = ctx.enter_context(tc.tile_pool(name="zps", bufs=2, space="PSUM"))
        rb_ps = ctx.enter_context(tc.tile_pool(name="rbps", bufs=1, space="PSUM"))
        mm_ps = ctx.enter_context(tc.tile_pool(name="mmps", bufs=2, space="PSUM"))

        # ---- resident constants ----
        # weights first (small, unblock the first projections), then x^T for
        # batch 0 split across two DMA queues, then batch 1.
        wq_sb = const.tile([P, NDC, HPC * DH], bf16)
        nc.sync.dma_start(wq_sb[:], wq_d.rearrange("(dc p) m -> p dc m", p=P))
        wk_sb = const.tile([P, NDC, HPC * DH], bf16)
        nc.gpsimd.dma_start(wk_sb[:], wk_d.rearrange("(dc p) m -> p dc m", p=P))
        wv_sb = const.tile([P, NDC, HPC * DH], bf16)
        nc.scalar.dma_start(wv_sb[:], wv_d.rearrange("(dc p) m -> p dc m", p=P))
        wo_sb = const.tile([HPC * DH, D], bf16)
        nc.scalar.dma_start(wo_sb[:], wo_d[:])
        bq_sb = const.tile([HPC * DH, 1], f32)
        nc.scalar.dma_start(bq_sb[:], bq_d[:])
        bk_sb = const.tile([HPC * DH, 1], f32)
        nc.scalar.dma_start(bk_sb[:], bk_d[:])
        msk_sb = const.tile([P, P], f32)
        nc.scalar.dma_start(msk_sb[:], msk_d[:])
        ones_sb = const.tile([1, DH], bf16)
        nc.vector.memset(ones_sb[:], 1.0)
        xt_sb = const.tile([P, B, NDC, S], bf16)
        half = NDC // 2
        for b in range(B):
            nc.sync.dma_start(
                xt_sb[:, b, 0:half, :],
                xT[b, 0:half * P, :].rearrange("(dc p) s -> p dc s", p=P))
            nc.gpsimd.dma_start(
                xt_sb[:, b, half:NDC, :],
                xT[b, half * P:NDC * P, :].rearrange("(dc p) s -> p dc s", p=P))

        qt = {}
        kt = {}
        vv = {}

        def qkv_chunk(b, t):
            """Q and K projection for q-tile t of batch b (PE-dense filler)."""
            for w_sb, dst, bias in ((wq_sb, qt[b], bq_sb),
                                    (wk_sb, kt[b], bk_sb)):
                ps = mm_ps.tile([P, QT_W], f32, tag="mm")
                for dc in range(NDC):
                    nc.tensor.matmul(
                        ps[:], w_sb[:, dc, :],
                        xt_sb[:, b, dc, t * QT_W:(t + 1) * QT_W],
                        start=(dc == 0), stop=(dc == NDC - 1))
                nc.vector.tensor_tensor(
                    dst[:, t * QT_W:(t + 1) * QT_W], ps[:],
                    bias[:].to_broadcast([P, QT_W]), ALU.add)

        def v_chunk(b, g):
            """V projection for s-blocks 2g..2g+1 of batch b (one copy)."""
            ps = mm_ps.tile([P, QT_W], f32, tag="mm")
            for i in range(2):
                sb = 2 * g + i
                for dc in range(NDC):
                    nc.tensor.matmul(
                        ps[:, i * P:i * P + HPC * DH],
                        xt_sb[:, b, dc, sb * P:(sb + 1) * P],
                        wv_sb[:, dc, :],
                        start=(dc == 0), stop=(dc == NDC - 1),
                        skip_group_check=True)
            nc.vector.tensor_copy(
                out=vv[b][:, 2 * g:2 * g + 2, :, 0:DH],
                in_=ps[:, 0:2 * P].rearrange("p (s h e) -> p s h e", h=HPC, e=DH))

        def attn_unit(b, h, t, zt_sb):
            """Scores + softmax + AV for one (batch, head, q-tile).

            Software-pipelined by 2: the PE queue sees S(kb+1), S(kb+2)
            ahead of AV(kb), so the exp wait never blocks score matmuls."""
            qt_sb, kt_sb, v_sb = qt[b], kt[b], vv[b]
            nkb = 4 * t + 4
            DEPTH = 2
            zps = z_ps.tile([P, QT_W], f32, tag="z")
            pending = []

            def emit_scores(kb):
                j = kb - 4 * t  # >=0 -> diagonal-region block
                width = QT_W - P * j if j >= 0 else QT_W
                qoff = P * j if j >= 0 else 0
                sps = st_ps.tile([P, QT_W], f32, tag="st")
                nc.tensor.matmul(
                    sps[:, 0:width],
                    kt_sb[h * DH:(h + 1) * DH, kb * P:(kb + 1) * P],
                    qt_sb[h * DH:(h + 1) * DH,
                          t * QT_W + qoff:(t + 1) * QT_W],
                    start=True, stop=True)
                if j >= 0:
                    nc.vector.tensor_tensor(
                        sps[:, 0:P], sps[:, 0:P], msk_sb[:], ALU.add)
                pt = pt_pool.tile([P, QT_W], bf16, tag="pt")
                nc.scalar.activation(pt[:, 0:width], sps[:, 0:width], AF.Exp)
                return (kb, pt, width, qoff)

            def emit_av(kb, pt, width, qoff):
                nc.tensor.matmul(
                    zps[0:DH + 1, qoff:QT_W],
                    v_sb[:, kb, h, :],
                    pt[:, 0:width],
                    start=(kb == 0), stop=(kb == nkb - 1),
                    skip_group_check=True)

            for kb in range(nkb):
                pending.append(emit_scores(kb))
                if len(pending) > DEPTH:
                    emit_av(*pending.pop(0))
            for item in pending:
                emit_av(*item)
            # normalize: Z = Z' * (1/rowsum); rowsum lives in zps row DH.
            # Broadcast rowsum over rows DH..127 of the same PSUM tile via a
            # K=1 matmul, then fast-reciprocal and multiply.
            rs_sb = sm_pool.tile([1, QT_W], bf16, tag="rs")
            nc.vector.tensor_copy(out=rs_sb[:], in_=zps[DH:DH + 1, :])
            rbps = rb_ps.tile([DH, QT_W], f32, tag="rb")
            nc.tensor.matmul(rbps[:], ones_sb[:], rs_sb[:],
                             start=True, stop=True)
            rc_sb = sm_pool.tile([DH, QT_W], f32, tag="rc")
            nc.vector.reciprocal_approx_fast(out=rc_sb[:], in_=rbps[:])
            nc.vector.tensor_tensor(
                zt_sb[h * DH:(h + 1) * DH, :], zps[0:DH, :], rc_sb[:],
                ALU.mult)

        def oproj(b, t, zt_sb):
            for c in range(QT_W // P):
                o_sb = o_pool.tile([P, D], bf16, tag="o")
                for half in range(2):
                    ops = mm_ps.tile([P, QT_W], f32, tag="mm")
                    nc.tensor.matmul(
                        ops[:], zt_sb[:, c * P:(c + 1) * P],
                        wo_sb[:, half * 512:(half + 1) * 512],
                        start=True, stop=True)
                    nc.any.tensor_copy(
                        out=o_sb[:, half * 512:(half + 1) * 512],
                        in_=ops[:])
                row0 = t * QT_W + c * P
                nc.sync.dma_start(out_d[b, row0:row0 + P, :], o_sb[:])

        for b in range(B):
            qt[b] = qk_pool.tile([P, S], bf16, tag="qt", name=f"qt{b}")
            kt[b] = qk_pool.tile([P, S], bf16, tag="qt", name=f"kt{b}")
            vv[b] = v_pool.tile([P, NKB, HPC, DH + 1], bf16, tag="v", name=f"v{b}")
            nc.vector.memset(vv[b][:, :, :, DH:DH + 1], 1.0)

        # batch 0 projections up front (dense PE work, warms HAM)
        for t in range(NQT):
            qkv_chunk(0, t)
        for g in range(NKB // 2):
            v_chunk(0, g)

        # batch-0 attention interleaved with batch-1 Q/K projections so the
        # PE has dense filler while ScalarE runs the exps
        for t in range(NQT):
            zt_sb = zt_pool.tile([P, QT_W], bf16, tag="zt")
            for h in range(HPC):
                u = t * HPC + h
                attn_unit(0, h, t, zt_sb)
                if u < NQT:
                    qkv_chunk(1, u)
                elif u == NQT:
                    v_chunk(1, 0)
                    v_chunk(1, 1)
            oproj(0, t, zt_sb)

        # batch-1 attention; remaining V blocks emitted just-in-time per
        # q-tile so they act as PE filler between the exp-bound units
        for t in range(NQT):
            if t > 0:
                v_chunk(1, 2 * t)
                v_chunk(1, 2 * t + 1)
            zt_sb = zt_pool.tile([P, QT_W], bf16, tag="zt")
            for h in range(HPC):
                attn_unit(1, h, t, zt_sb)
            oproj(1, t, zt_sb)

    nc.compile()
    return nc


def _prep_in_maps(inputs):
    x = np.asarray(inputs["x"], dtype=np.float32)
    xT = np.ascontiguousarray(x.transpose(0, 2, 1)).astype(BF16)  # [B, D, S]
    W_Q = np.asarray(inputs["W_Q"], dtype=np.float32)
    W_K = np.asarray(inputs["W_K"], dtype=np.float32)
    W_V = np.asarray(inputs["W_V"], dtype=np.float32)
    W_O = np.asarray(inputs["W_O"], dtype=np.float32)
    b_Q = np.asarray(inputs["b_Q"], dtype=np.float32)
    b_K = np.asarray(inputs["b_K"], dtype=np.float32)
    scale = 1.0 / np.sqrt(DH)
    msk = np.where(np.arange(P)[:, None] <= np.arange(P)[None, :],
                   np.float32(0.0), np.float32(NEG)).astype(np.float32)
    in_maps = []
    for c in range(NCORES):
        hs = [HPC * c + i for i in range(HPC)]
        wq = np.concatenate([W_Q[h] for h in hs], axis=1) * scale
        wk = np.concatenate([W_K[h] for h in hs], axis=1)
        wv = np.concatenate([W_V[h] for h in hs], axis=1)
        wo = np.concatenate([W_O[h] for h in hs], axis=0)
        bq = np.concatenate([b_Q[h] for h in hs])[:, None] * scale
        bk = np.concatenate([b_K[h] for h in hs])[:, None]
        in_maps.append({
            "xT": xT,
            "wq": np.ascontiguousarray(wq).astype(BF16),
            "wk": np.ascontiguousarray(wk).astype(BF16),
            "wv": np.ascontiguousarray(wv).astype(BF16),
            "wo": np.ascontiguousarray(wo).astype(BF16),
            "bq": bq.astype(np.float32),
            "bk": bk.astype(np.float32),
            "msk": msk,
        })
    return in_maps


def _run(inputs, trace=False, trace_cores=None):
    from concourse.bass_utils import run_bass_kernel_spmd

    if "nc" not in _CACHE:
        _CACHE["nc"] = _build_nc()
    nc = _CACHE["nc"]
    in_maps = _prep_in_maps(inputs)
    res = run_bass_kernel_spmd(
        nc, in_maps, core_ids=list(range(NCORES)),
        trace=trace, trace_cores=trace_cores)

    out = np.zeros((B, S, D), dtype=np.float32)
    for c in range(NCORES):
        out += res.results[c]["out"].astype(np.float32)
    # exact host fold of the zero-pattern-sum bias terms:
    # z includes +b_V per head -> out += sum_h b_V[h] @ W_O[h]; plus b_O.
    b_V = np.asarray(inputs["b_V"], dtype=np.float32)
    W_O = np.asarray(inputs["W_O"], dtype=np.float32)
    b_O = np.asarray(inputs["b_O"], dtype=np.float32)
    out += np.einsum("he,hed->d", b_V, W_O) + b_O

    residual = np.asarray(inputs["residual"], dtype=np.float32)
    return (residual, out), res


def kernel(**inputs):
    (residual, out), _ = _run(inputs, trace=False)
    return residual, out


# revision 29
# speedup vs baseline: 1.0641x; 1.0248x over previous
"""Multi-head causal attention (B=2,S=2048,D=1024,H=16,DH=64) on 8 TRN2 cores.

Sharding: 2 heads per core (tensor parallel). Each core computes QKV for its
2 heads from the full x, causal attention, and its partial of the output
projection [B,S,D]. The host sums the 8 partials (the W_O head-sum).

On-device layouts (matmul contracts over the partition dim):
  QT/KT  [2*DH=128 part, S]   (heads stacked on partitions; 1/sqrt(DH) folded into W_Q)
  V      [S part (128-blocks), heads, DH+1]  (ones column -> softmax row-sums for free)
  S^T    [k 128 part, q 512]  per (k-block, q-tile); above-diagonal blocks skipped
  Z'^T   [DH+1 part, q 512]   accumulated over k-blocks; row DH = exp row-sum
  out    partial [B,S,D] bf16, summed across cores on host
"""

import os
import sys

import numpy as np

if "/opt/trn_rl_repo" not in sys.path:
    sys.path.insert(0, "/opt/trn_rl_repo")

import ml_dtypes

B, S, D, H, DH = 2, 2048, 1024, 16, 64
NCORES = 8
HPC = H // NCORES          # heads per core
P = 128
QT_W = 512                 # q-tile width
NQT = S // QT_W            # 4 q-tiles
NKB = S // P               # 16 k-blocks
NDC = D // P               # 8 contraction chunks for projections
NEG = -1.0e5

BF16 = ml_dtypes.bfloat16

_CACHE = {}


def _build_nc(B=B, S=S, D=D, HPC=HPC, DH=DH):
    import concourse.tile as tile
    import concourse.mybir as mybir
    from concourse import bacc
    from concourse import masks
    from contextlib import ExitStack

    QT_W = 512
    NQT = S // QT_W
    NKB = S // P
    NDC = D // P

    f32 = mybir.dt.float32
    bf16 = mybir.dt.bfloat16
    AF = mybir.ActivationFunctionType
    ALU = mybir.AluOpType

    nc = bacc.Bacc("TRN2", target_bir_lowering=False, debug=False,
                   num_devices=NCORES)

    xT = nc.dram_tensor("xT", [B, D, S], bf16, kind="ExternalInput").ap()
    wq_d = nc.dram_tensor("wq", [D, HPC * DH], bf16, kind="ExternalInput").ap()
    wk_d = nc.dram_tensor("wk", [D, HPC * DH], bf16, kind="ExternalInput").ap()
    wv_d = nc.dram_tensor("wv", [D, HPC * DH], bf16, kind="ExternalInput").ap()
    wo_d = nc.dram_tensor("wo", [HPC * DH, D], bf16, kind="ExternalInput").ap()
    bq_d = nc.dram_tensor("bq", [HPC * DH, 1], f32, kind="ExternalInput").ap()
    bk_d = nc.dram_tensor("bk", [HPC * DH, 1], f32, kind="ExternalInput").ap()
    msk_d = nc.dram_tensor("msk", [P, P], f32, kind="ExternalInput").ap()
    out_d = nc.dram_tensor("out", [B, S, D], bf16, kind="ExternalOutput").ap()

    with tile.TileContext(nc) as tc, ExitStack() as ctx:
        const = ctx.enter_context(tc.tile_pool(name="const", bufs=1))
        qk_pool = ctx.enter_context(tc.tile_pool(name="qk", bufs=4))
        v_pool = ctx.enter_context(tc.tile_pool(name="v", bufs=2))
        pt_pool = ctx.enter_context(tc.tile_pool(name="pt", bufs=8))
        sm_pool = ctx.enter_context(tc.tile_pool(name="sm", bufs=6))
        zt_pool = ctx.enter_context(tc.tile_pool(name="zt", bufs=4))
        o_pool = ctx.enter_context(tc.tile_pool(name="o", bufs=3))
        st_ps = ctx.enter_context(tc.tile_pool(name="stps", bufs=3, space="PSUM"))
        z_ps = ctx.enter_context(tc.tile_pool(name="zps", bufs=2, space="PSUM"))
        rb_ps = ctx.enter_context(tc.tile_pool(name="rbps", bufs=1, space="PSUM"))
        mm_ps = ctx.enter_context(tc.tile_pool(name="mmps", bufs=2, space="PSUM"))

        # ---- resident constants ----
        # weights first (small, unblock the first projections), then x^T for
        # batch 0 split across two DMA queues, then batch 1.
        wq_sb = const.tile([P, NDC, HPC * DH], bf16)
        nc.sync.dma_start(wq_sb[:], wq_d.rearrange("(dc p) m -> p dc m", p=P))
        wk_sb = const.tile([P, NDC, HPC * DH], bf16)
        nc.gpsimd.dma_start(wk_sb[:], wk_d.rearrange("(dc p) m -> p dc m", p=P))
        wv_sb = const.tile([P, NDC, HPC * DH], bf16)
        nc.scalar.dma_start(wv_sb[:], wv_d.rearrange("(dc p) m -> p dc m", p=P))
        wo_sb = const.tile([HPC * DH, D], bf16)
        nc.scalar.dma_start(wo_sb[:], wo_d[:])
        bq_sb = const.tile([HPC * DH, 1], f32)
        nc.scalar.dma_start(bq_sb[:], bq_d[:])
        bk_sb = const.tile([HPC * DH, 1], f32)
        nc.scalar.dma_start(bk_sb[:], bk_d[:])
        msk_sb = const.tile([P, P], f32)
        nc.scalar.dma_start(msk_sb[:], msk_d[:])
        ones_sb = const.tile([1, DH], bf16)
        nc.vector.memset(ones_sb[:], 1.0)
        xt_sb = const.tile([P, B, NDC, S], bf16)
        half = NDC // 2
        for b in range(B):
            nc.sync.dma_start(
                xt_sb[:, b, 0:half, :],
                xT[b, 0:half * P, :].rearrange("(dc p) s -> p dc s", p=P))
            nc.gpsimd.dma_start(
                xt_sb[:, b, half:NDC, :],
                xT[b, half * P:NDC * P, :].rearrange("(dc p) s -> p dc s", p=P))

        qt = {}
        kt = {}
        vv = {}

        def qkv_chunk(b, t):
            """Q and K projection for q-tile t of batch b (PE-dense filler)."""
            for w_sb, dst, bias in ((wq_sb, qt[b], bq_sb),
                                    (wk_sb, kt[b], bk_sb)):
                ps = mm_ps.tile([P, QT_W], f32, tag="mm")
                for dc in range(NDC):
                    nc.tensor.matmul(
                        ps[:], w_sb[:, dc, :],
                        xt_sb[:, b, dc, t * QT_W:(t + 1) * QT_W],
                        start=(dc == 0), stop=(dc == NDC - 1))
                nc.vector.tensor_tensor(
                    dst[:, t * QT_W:(t + 1) * QT_W], ps[:],
                    bias[:].to_broadcast([P, QT_W]), ALU.add)

        def v_chunk(b, g):
            """V projection for s-blocks 2g..2g+1 of batch b (one copy)."""
            ps = mm_ps.tile([P, QT_W], f32, tag="mm")
            for i in range(2):
                sb = 2 * g + i
                for dc in range(NDC):
                    nc.tensor.matmul(
                        ps[:, i * P:i * P + HPC * DH],
                        xt_sb[:, b, dc, sb * P:(sb + 1) * P],
                        wv_sb[:, dc, :],
                        start=(dc == 0), stop=(dc == NDC - 1),
                        skip_group_check=True)
            nc.vector.tensor_copy(
                out=vv[b][:, 2 * g:2 * g + 2, :, 0:DH],
                in_=ps[:, 0:2 * P].rearrange("p (s h e) -> p s h e", h=HPC, e=DH))

        def attn_unit(b, h, t, zt_sb):
            """Scores + softmax + AV for one (batch, head, q-tile).

            Software-pipelined by 2: the PE queue sees S(kb+1), S(kb+2)
            ahead of AV(kb), so the exp wait never blocks score matmuls."""
            qt_sb, kt_sb, v_sb = qt[b], kt[b], vv[b]
            nkb = 4 * t + 4
            DEPTH = 2
            zps = z_ps.tile([P, QT_W], f32, tag="z")
            pending = []

            def emit_scores(kb):
                j = kb - 4 * t  # >=0 -> diagonal-region block
                width = QT_W - P * j if j >= 0 else QT_W
                qoff = P * j if j >= 0 else 0
                sps = st_ps.tile([P, QT_W], f32, tag="st")
                nc.tensor.matmul(
                    sps[:, 0:width],
                    kt_sb[h * DH:(h + 1) * DH, kb * P:(kb + 1) * P],
                    qt_sb[h * DH:(h + 1) * DH,
                          t * QT_W + qoff:(t + 1) * QT_W],
                    start=True, stop=True)
                if j >= 0:
                    nc.vector.tensor_tensor(
                        sps[:, 0:P], sps[:, 0:P], msk_sb[:], ALU.add)
                pt = pt_pool.tile([P, QT_W], bf16, tag="pt")
                nc.scalar.activation(pt[:, 0:width], sps[:, 0:width], AF.Exp)
                return (kb, pt, width, qoff)

            def emit_av(kb, pt, width, qoff):
                nc.tensor.matmul(
                    zps[0:DH + 1, qoff:QT_W],
                    v_sb[:, kb, h, :],
                    pt[:, 0:width],
                    start=(kb == 0), stop=(kb == nkb - 1),
                    skip_group_check=True)

            for kb in range(nkb):
                pending.append(emit_scores(kb))
                if len(pending) > DEPTH:
                    emit_av(*pending.pop(0))
            for item in pending:
                emit_av(*item)
            # normalize: Z = Z' * (1/rowsum); rowsum lives in zps row DH.
            # Broadcast rowsum over rows DH..127 of the same PSUM tile via a
            # K=1 matmul, then fast-reciprocal and multiply.
            rs_sb = sm_pool.tile([1, QT_W], bf16, tag="rs")
            nc.vector.tensor_copy(out=rs_sb[:], in_=zps[DH:DH + 1, :])
            rbps = rb_ps.tile([DH, QT_W], f32, tag="rb")
            nc.tensor.matmul(rbps[:], ones_sb[:], rs_sb[:],
                             start=True, stop=True)
            rc_sb = sm_pool.tile([DH, QT_W], f32, tag="rc")
            nc.vector.reciprocal_approx_fast(out=rc_sb[:], in_=rbps[:])
            nc.vector.tensor_tensor(
                zt_sb[h * DH:(h + 1) * DH, :], zps[0:DH, :], rc_sb[:],
                ALU.mult)

        def oproj(b, t, zt_sb):
            for c in range(QT_W // P):
                o_sb = o_pool.tile([P, D], bf16, tag="o")
                for half in range(2):
                    ops = mm_ps.tile([P, QT_W], f32, tag="mm")
                    nc.tensor.matmul(
                        ops[:], zt_sb[:, c * P:(c + 1) * P],
                        wo_sb[:, half * 512:(half + 1) * 512],
                        start=True, stop=True)
                    nc.any.tensor_copy(
                        out=o_sb[:, half * 512:(half + 1) * 512],
                        in_=ops[:])
                row0 = t * QT_W + c * P
                nc.sync.dma_start(out_d[b, row0:row0 + P, :], o_sb[:])

        for b in range(B):
            qt[b] = qk_pool.tile([P, S], bf16, tag="qt", name=f"qt{b}")
            kt[b] = qk_pool.tile([P, S], bf16, tag="qt", name=f"kt{b}")
            vv[b] = v_pool.tile([P, NKB, HPC, DH + 1], bf16, tag="v", name=f"v{b}")
            nc.vector.memset(vv[b][:, :, :, DH:DH + 1], 1.0)

        # minimal upfront phase: batch-0 Q/K plus the first 4 V blocks
        for t in range(NQT):
            qkv_chunk(0, t)
        v_chunk(0, 0)
        v_chunk(0, 1)

        # batch-0 attention with just-in-time batch-0 V blocks and batch-1
        # projections as PE filler between the exp-bound units
        for t in range(NQT):
            if t > 0:
                v_chunk(0, 2 * t)
                v_chunk(0, 2 * t + 1)
            zt_sb = zt_pool.tile([P, QT_W], bf16, tag="zt")
            for h in range(HPC):
                u = t * HPC + h
                attn_unit(0, h, t, zt_sb)
                if u < NQT:
                    qkv_chunk(1, u)
                elif u - NQT < NKB // 2:
                    v_chunk(1, u - NQT)
            oproj(0, t, zt_sb)

        # batch-1 attention; remaining V blocks emitted just-in-time per
        # q-tile so they act as PE filler between the exp-bound units
        v1_done = min(NQT * HPC - NQT, NKB // 2)
        for t in range(NQT):
            while v1_done < min(2 * t + 2, NKB // 2):
                v_chunk(1, v1_done)
                v1_done += 1
            zt_sb = zt_pool.tile([P, QT_W], bf16, tag="zt")
            for h in range(HPC):
                attn_unit(1, h, t, zt_sb)
            oproj(1, t, zt_sb)

    nc.compile()
    return nc


def _prep_in_maps(inputs):
    x = np.asarray(inputs["x"], dtype=np.float32)
    xT = np.ascontiguousarray(x.transpose(0, 2, 1)).astype(BF16)  # [B, D, S]
    W_Q = np.asarray(inputs["W_Q"], dtype=np.float32)
    W_K = np.asarray(inputs["W_K"], dtype=np.float32)
    W_V = np.asarray(inputs["W_V"], dtype=np.float32)
    W_O = np.asarray(inputs["W_O"], dtype=np.float32)
    b_Q = np.asarray(inputs["b_Q"], dtype=np.float32)
    b_K = np.asarray(inputs["b_K"], dtype=np.float32)
    scale = 1.0 / np.sqrt(DH)
    msk = np.where(np.arange(P)[:, None] <= np.arange(P)[None, :],
                   np.float32(0.0), np.float32(NEG)).astype(np.float32)
    in_maps = []
    for c in range(NCORES):
        hs = [HPC * c + i for i in range(HPC)]
        wq = np.concatenate([W_Q[h] for h in hs], axis=1) * scale
        wk = np.concatenate([W_K[h] for h in hs], axis=1)
        wv = np.concatenate([W_V[h] for h in hs], axis=1)
        wo = np.concatenate([W_O[h] for h in hs], axis=0)
        bq = np.concatenate([b_Q[h] for h in hs])[:, None] * scale
        bk = np.concatenate([b_K[h] for h in hs])[:, None]
        in_maps.append({
            "xT": xT,
            "wq": np.ascontiguousarray(wq).astype(BF16),
            "wk": np.ascontiguousarray(wk).astype(BF16),
            "wv": np.ascontiguousarray(wv).astype(BF16),
            "wo": np.ascontiguousarray(wo).astype(BF16),
            "bq": bq.astype(np.float32),
            "bk": bk.astype(np.float32),
            "msk": msk,
        })
    return in_maps


def _run(inputs, trace=False, trace_cores=None):
    from concourse.bass_utils import run_bass_kernel_spmd

    if "nc" not in _CACHE:
        _CACHE["nc"] = _build_nc()
    nc = _CACHE["nc"]
    in_maps = _prep_in_maps(inputs)
    res = run_bass_kernel_spmd(
        nc, in_maps, core_ids=list(range(NCORES)),
        trace=trace, trace_cores=trace_cores)

    out = np.zeros((B, S, D), dtype=np.float32)
    for c in range(NCORES):
        out += res.results[c]["out"].astype(np.float32)
    # exact host fold of the zero-pattern-sum bias terms:
    # z includes +b_V per head -> out += sum_h b_V[h] @ W_O[h]; plus b_O.
    b_V = np.asarray(inputs["b_V"], dtype=np.float32)
    W_O = np.asarray(inputs["W_O"], dtype=np.float32)
    b_O = np.asarray(inputs["b_O"], dtype=np.float32)
    out += np.einsum("he,hed->d", b_V, W_O) + b_O

    residual = np.asarray(inputs["residual"], dtype=np.float32)
    return (residual, out), res


def kernel(**inputs):
    (residual, out), _ = _run(inputs, trace=False)
    return residual, out
